# revision 2
# baseline (speedup 1.0000x reference)
"""Multi-head attention (B=4, N=2048, E=512, H=8) on 8 TRN2 NeuronCores — v2.

Sharding: data-parallel over (batch x query-half), as v1: core c handles batch
c//2, query rows [(c%2)*1024, ...+1024) of the rotated sequence; K/V for the
full 2048-key sequence are recomputed per core (no collectives).

v2 rebalances the engines (cost model: PE charges J-cycles per pass, ACT/DVE
charge free-size cycles):
  * all matmul operands bf16 (1 cycle/row; half the DMA + SBUF of f32r)
  * PV flipped to the natural orientation: stationary es [128k x 128q],
    moving V [128k, 65] (64 d + ones col for the denominator) -> out
    [128 q, 65] PSUM, accumulated over 16 key tiles.  66.5k J-cycles vs
    131k in the v1 O^T orientation (which filled only 65/128 out rows).
  * softmax normalization fused into the PV drain: per-partition
    tensor_scalar multiply by 1/den (denominator = PSUM col 64), f32 PSUM
    -> bf16 SBUF in one DVE op.
  * O^T for the projection rebuilt with PE transposes ([128,128] bf16
    blocks via identity, 512 J-cycles/unit).
  * exp split: ACT (table exp, 13/16 of key tiles) + DVE (1-op Schraudolph
    bit-trick exp -> int16 viewed as bf16, 3/16) so the 131k-elem exp
    stream stops being the single-engine wall.  Schraudolph rel err ~2-4%
    on 3/16 of the weights adds ~5e-3 end-to-end (gate 2e-2).
  * QKV biases folded into the PSUM->bf16 drains (per-partition scalar for
    Q/K; broadcast tile for V's free-axis bias); proj bias in the Y drain.

PE ~300k cycles (~125us) is the design bottleneck; ACT ~113us and DVE
~80us ride in its shadow.
"""

import sys

for _p in ("/opt/trn_rl_repo",):
    if _p not in sys.path:
        sys.path.insert(0, _p)

import numpy as np

import concourse.bass as bass
import concourse.bacc as bacc
import concourse.tile as tile
import concourse.mybir as mybir
from concourse.bass_utils import run_bass_kernel_spmd


def _stub_axon_hooks():
    """Some axon client installs lack antenv.axon_hooks; stub it so
    run_bass_kernel_spmd(trace=True) degrades gracefully."""
    import types

    try:
        import antenv
    except ImportError:
        return
    try:
        from antenv import axon_hooks  # noqa: F401
        return
    except ImportError:
        pass
    mod = types.ModuleType("antenv.axon_hooks")
    mod.get_axon_ntff_profile_hook = lambda: None
    sys.modules["antenv.axon_hooks"] = mod
    antenv.axon_hooks = mod


_stub_axon_hooks()

F32 = mybir.dt.float32
BF16 = mybir.dt.bfloat16
I16 = mybir.dt.int16
EXP = mybir.ActivationFunctionType.Exp
MUL = mybir.AluOpType.mult
ADD = mybir.AluOpType.add

E = 512          # embedding
N = 2048         # sequence length (per batch)
NQ = 1024        # queries handled per core
H = 8            # heads
D = 64           # head dim
EC = E // 128    # 4 contraction chunks of 128
NT = N // 128    # 16 key m-tiles
SCALE = D ** -0.5

LOG2E = 1.4426950408889634
A_SCH = 128.0 * LOG2E * SCALE     # folds the softmax scale into the trick
B_SCH = 16256.0 - 4.75            # 127*128 - C (C tuned for min rel err)

# key tiles whose exp runs on DVE (Schraudolph) instead of ACT — kept away
# from the unit seams, where the DVE must turn the PV drains around fast
SCH_M = frozenset((4, 8, 12))


def emit(nc, tc, ctx, dram):
    (xT_d, wq_d, qb_d, vb_d, pw_d, pb_d, ones_d, zb_d, idn_d, out_d) = dram
    ctx.enter_context(
        nc.allow_low_precision("bf16 matmul pipeline; drains round to bf16")
    )

    big = ctx.enter_context(tc.tile_pool(name="big", bufs=1))
    qkp = ctx.enter_context(tc.tile_pool(name="qkp", bufs=2, space="PSUM"))
    sgp = ctx.enter_context(tc.tile_pool(name="sgp", bufs=2, space="PSUM"))
    pvp = ctx.enter_context(tc.tile_pool(name="pvp", bufs=2, space="PSUM"))
    esp = ctx.enter_context(tc.tile_pool(name="esp", bufs=6))
    osp = ctx.enter_context(tc.tile_pool(name="osp", bufs=8))
    yop = ctx.enter_context(tc.tile_pool(name="yop", bufs=4))
    rdp = ctx.enter_context(tc.tile_pool(name="rdp", bufs=4))

    # ---- persistent SBUF tiles ----
    xT = [big.tile([128, N], BF16, name=f"xT{e}") for e in range(EC)]
    wq = [big.tile([128, 3 * E], BF16, name=f"wq{e}") for e in range(EC)]
    pw = [big.tile([128, E], BF16, name=f"pw{t}") for t in range(4)]
    QT = [big.tile([128, NQ], BF16, name=f"QT{t}") for t in range(4)]
    KT = [big.tile([128, N], BF16, name=f"KT{t}") for t in range(4)]
    VA = [big.tile([128, 8 * 65], BF16, name=f"VA{m}") for m in range(NT)]
    OT = [big.tile([128, NQ], BF16, name=f"OT{t}") for t in range(4)]
    qkb = big.tile([128, 12], F32, name="qkb")   # qkv bias cols, 12 x 128
    pbt = big.tile([128, 4], F32, name="pbt")    # proj bias cols
    ones_row = big.tile([1, 128], BF16, name="ones_row")
    vbr = big.tile([1, E], BF16, name="vbr")
    vbb = big.tile([128, E], BF16, name="vbb")
    zb = big.tile([128, 1], F32, name="zb")
    qb = [qkb[:, t : t + 1] for t in range(4)]
    kb = [qkb[:, 4 + t : 4 + t + 1] for t in range(4)]
    pb = [pbt[:, t : t + 1] for t in range(4)]

    # ---- input DMAs ----
    # Two parallel descriptor-gen paths: SP HWDGE for wq/consts, Pool SWDGE
    # for the xT sweep.  Critical path first on each.
    for e in range(EC):
        nc.sync.dma_start(
            wq[e][:, 0:128], wq_d[128 * e : 128 * (e + 1), 0:128]
        )
    nc.sync.dma_start(zb[:], zb_d[:])
    # all qkv bias columns in one strided DMA: col j holds rows 128j..128j+128
    nc.sync.dma_start(
        qkb[:], qb_d[:, 0:1].rearrange("(c p) one -> p (c one)", p=128)
    )
    nc.sync.dma_start(vbr[:], vb_d[:])
    nc.sync.dma_start(ones_row[:], ones_d[:])
    # K pair-0 cols
    for e in range(EC):
        nc.sync.dma_start(
            wq[e][:, 512 : 512 + 128],
            wq_d[128 * e : 128 * (e + 1), 512 : 512 + 128],
        )
    # V cols
    for e in range(EC):
        nc.sync.dma_start(
            wq[e][:, 1024:1536], wq_d[128 * e : 128 * (e + 1), 1024:1536]
        )
    # Q pairs 1-3, K pairs 1-3
    for e in range(EC):
        nc.sync.dma_start(
            wq[e][:, 128:512], wq_d[128 * e : 128 * (e + 1), 128:512]
        )
    for e in range(EC):
        nc.sync.dma_start(
            wq[e][:, 640:1024], wq_d[128 * e : 128 * (e + 1), 640:1024]
        )
    nc.sync.dma_start(
        pbt[:], pb_d[:, 0:1].rearrange("(c p) one -> p (c one)", p=128)
    )
    for t in range(4):
        nc.sync.dma_start(pw[t][:], pw_d[128 * t : 128 * (t + 1), :])
    # xT sweep on the Pool SWDGE path
    for e in range(EC):
        nc.gpsimd.dma_start(
            xT[e][:, 0:512], xT_d[128 * e : 128 * (e + 1), 0:512]
        )
    for e in range(EC):
        nc.gpsimd.dma_start(
            xT[e][:, 512:1024], xT_d[128 * e : 128 * (e + 1), 512:1024]
        )
    for e in range(EC):
        nc.gpsimd.dma_start(
            xT[e][:, 1024:2048], xT_d[128 * e : 128 * (e + 1), 1024:2048]
        )

    # ones columns of VA are constant; fill them once on the Pool engine
    for m in range(NT):
        va3 = VA[m][:].rearrange("p (h c) -> p h c", c=65)
        nc.gpsimd.memset(va3[:, :, 64:65], 1.0)
    # identity for the tail PE transpose (loaded late, off the critical path)
    idn = big.tile([128, 128], BF16, name="idn")
    nc.sync.dma_start(idn[:], idn_d[:])
    # warm the exp table while input DMAs run
    zpre = big.tile([128, 1], F32, name="zpre")
    nc.scalar.activation(zpre[:], zb[:], EXP, bias=zb[:], scale=1.0)

    def emit_vbb():
        # V bias broadcast to all partitions (K=1 matmul), drained to bf16
        vbps = qkp.tile([128, 512], F32, tag="qk", name="vbps")
        nc.tensor.matmul(
            vbps[:], ones_row[0:1, :], vbr[:], start=True, stop=True
        )
        nc.vector.tensor_copy(vbb[:], vbps[:])

    # ================= QKV emission =================
    # Q/K: features on partitions (stationary = wq column slice).
    def emit_q(t, c):
        ps = qkp.tile([128, 512], F32, tag="qk", name="psq")
        for e in range(EC):
            nc.tensor.matmul(
                ps[:],
                wq[e][:, 128 * t : 128 * (t + 1)],
                xT[e][:, 512 * c : 512 * (c + 1)],
                start=(e == 0),
                stop=(e == EC - 1),
            )
        nc.vector.tensor_scalar(
            QT[t][:, 512 * c : 512 * (c + 1)], ps[:], qb[t][:], None, ADD
        )

    def emit_k(t, c):
        ps = qkp.tile([128, 512], F32, tag="qk", name="psk")
        for e in range(EC):
            nc.tensor.matmul(
                ps[:],
                wq[e][:, 512 + 128 * t : 512 + 128 * (t + 1)],
                xT[e][:, 512 * c : 512 * (c + 1)],
                start=(e == 0),
                stop=(e == EC - 1),
            )
        nc.vector.tensor_scalar(
            KT[t][:, 512 * c : 512 * (c + 1)], ps[:], kb[t][:], None, ADD
        )

    # V: keys on partitions (stationary = xT token slice), bias via the
    # vbb broadcast tile during the bf16 drain.
    def emit_v(m):
        ps = qkp.tile([128, 512], F32, tag="qk", name="psv")
        for e in range(EC):
            nc.tensor.matmul(
                ps[:],
                xT[e][:, 128 * m : 128 * (m + 1)],
                wq[e][:, 1024:1536],
                start=(e == 0),
                stop=(e == EC - 1),
            )
        va3 = VA[m][:].rearrange("p (h c) -> p h c", c=65)
        nc.vector.tensor_add(
            va3[:, :, 0:64],
            ps[:].rearrange("p (h c) -> p h c", c=64),
            vbb[:].rearrange("p (h c) -> p h c", c=64),
        )

    # ================= attention =================
    # Unit (t, c2): head pair (2t, 2t+1), query block c2 (512 queries).
    # S natural: out [128 keys, 1024 = 512q x 2 heads] PSUM per m.
    # exp -> es bf16 SBUF.  PV flipped: stationary es [128k, 128q] slice,
    # moving VA head slice [128k, 65] -> out [128 q, 65] in the unit's
    # per-head PV accumulator [128, 260] (4 qc x 65).

    def emit_att_unit(t, c2, extras=None, lag=1, split_drain=False):
        # PV(m) is emitted `lag` m-steps after S(m)/exp(m) so the in-order PE
        # queue never waits on exp latency (or, in unit 0, on the V loads).
        nbase = 512 * c2
        pv = [pvp.tile([128, 260], F32, tag="pv", name=f"pv{h}") for h in (0, 1)]
        esq = {}
        for mm in range(NT + lag):
            if mm < NT:
                m = mm
                sg = sgp.tile([128, 1024], F32, tag="sg", name="sg")
                nc.tensor.matmul(
                    sg[:, 0:512],
                    KT[t][0:64, 128 * m : 128 * (m + 1)],
                    QT[t][0:64, nbase : nbase + 512],
                    start=True,
                    stop=True,
                )
                nc.tensor.matmul(
                    sg[:, 512:1024],
                    KT[t][64:128, 128 * m : 128 * (m + 1)],
                    QT[t][64:128, nbase : nbase + 512],
                    start=True,
                    stop=True,
                )
            if extras is not None and mm in extras:
                extras[mm]()
            if mm < NT:
                es = esp.tile([128, 1024], BF16, tag="es", name="es")
                if mm in SCH_M:
                    nc.vector.tensor_scalar(
                        es[:].bitcast(I16), sg[:], A_SCH, B_SCH, MUL, ADD
                    )
                else:
                    nc.scalar.activation(
                        es[:], sg[:], EXP, bias=zb[:], scale=SCALE
                    )
                esq[mm] = es
            if mm >= lag:
                m = mm - lag
                es = esq.pop(m)
                for h in (0, 1):
                    for qc in range(4):
                        # start_tensor_calc resets the whole PSUM bank on
                        # real HW: exactly ONE start per pv bank (m0,qc0);
                        # the other qc groups accumulate onto the zeroed
                        # bank (skip the sim's per-region group check)
                        nc.tensor.matmul(
                            pv[h][:, 65 * qc : 65 * qc + 65],
                            es[:, 512 * h + 128 * qc : 512 * h + 128 * qc + 128],
                            VA[m][:, 65 * (2 * t + h) : 65 * (2 * t + h) + 65],
                            start=(m == 0 and qc == 0),
                            stop=(m == NT - 1),
                            skip_group_check=(m == 0 and qc > 0),
                        )
        # drain: per-head reciprocal of the 4 denominator cols, then 8
        # per-partition-scaled copies f32 PSUM -> bf16 SBUF (transpose
        # input).  split_drain routes one head's copies through ACT (used
        # for the last unit, where ACT is otherwise finished).
        osc = [
            osp.tile([128, 128], BF16, tag="osc", name=f"osc{qc}")
            for qc in range(4)
        ]
        rcps = []
        for h in (0, 1):
            rcp = rdp.tile([128, 4], F32, tag="rd", name="rcp")
            den = pv[h][:].rearrange("p (q c) -> p q c", c=65)[:, :, 64:65]
            nc.vector.reciprocal(rcp[:], den[:, :, 0])
            rcps.append(rcp)
        for qc in range(4):
            for h in (0, 1):
                if split_drain and h == 1:
                    nc.scalar.activation(
                        osc[qc][:, 64 : 128],
                        pv[1][:, 65 * qc : 65 * qc + 64],
                        mybir.ActivationFunctionType.Copy,
                        bias=0.0,
                        scale=rcps[1][:, qc : qc + 1],
                    )
                else:
                    nc.vector.tensor_scalar(
                        osc[qc][:, 64 * h : 64 * h + 64],
                        pv[h][:, 65 * qc : 65 * qc + 64],
                        rcps[h][:, qc : qc + 1],
                        None,
                        MUL,
                    )
        return osc

    def emit_o_transpose(t, c2, osc, tag="qk"):
        # PE transposes via identity into one PSUM ring tile + one DVE copy
        # out.  start_tensor_calc resets the whole bank on real HW, so only
        # the first transpose starts; the rest accumulate onto the zeroed
        # bank.  Mid-kernel: qk ring (emitted from the NEXT unit's extras);
        # tail: sg ring (the S stream is done by then).
        pool = sgp if tag == "sg" else qkp
        trp = pool.tile([128, 512], BF16, tag=tag, name="trp")
        for qc in range(4):
            nc.tensor.matmul(
                trp[:, 128 * qc : 128 * qc + 128],
                osc[qc][:],
                idn[:],
                is_transpose=True,
                start=(qc == 0),
                stop=True,
                skip_group_check=(qc > 0),
            )
        nc.vector.tensor_copy(OT[t][:, 512 * c2 : 512 * (c2 + 1)], trp[:])

    proj_ps = {}

    def emit_proj_start(o, c2, nt, tag="qk"):
        pool = sgp if tag == "sg" else qkp
        ps = pool.tile([128, 512], F32, tag=tag, name="psy")
        proj_ps[(o, c2)] = ps
        for td in range(nt):
            nc.tensor.matmul(
                ps[:],
                pw[td][:, 128 * o : 128 * (o + 1)],
                OT[td][:, 512 * c2 : 512 * (c2 + 1)],
                start=(td == 0),
                stop=False,
            )

    def emit_proj_cont(o, c2, td):
        ps = proj_ps[(o, c2)]
        nc.tensor.matmul(
            ps[:],
            pw[td][:, 128 * o : 128 * (o + 1)],
            OT[td][:, 512 * c2 : 512 * (c2 + 1)],
            start=False,
            stop=False,
        )

    COPY = mybir.ActivationFunctionType.Copy

    def emit_proj_finish(o, c2, nt, drain="dve", store="hw"):
        ps = proj_ps.pop((o, c2))
        for td in range(nt, 4):
            nc.tensor.matmul(
                ps[:],
                pw[td][:, 128 * o : 128 * (o + 1)],
                OT[td][:, 512 * c2 : 512 * (c2 + 1)],
                start=False,
                stop=(td == 3),
            )
        yo = yop.tile([128, 512], F32, tag="yo", name="yo")
        if drain == "act":
            nc.scalar.add(yo[:], ps[:], pb[o])
        else:
            nc.vector.tensor_scalar(yo[:], ps[:], pb[o], None, ADD)
        eng = nc.gpsimd if store == "sw" else nc.sync
        eng.dma_start(
            out_d[128 * o : 128 * (o + 1), 512 * c2 : 512 * (c2 + 1)], yo[:]
        )

    def emit_proj(o, c2, tag="qk", drain="dve", store="hw"):
        emit_proj_start(o, c2, 3, tag=tag)
        emit_proj_finish(o, c2, 3, drain=drain, store=store)

    # ================= schedule =================
    # K/V/Q for later units are slotted into earlier units' m-steps so PE
    # never waits on the QKV phase; transposes and proj ride at unit
    # boundaries.  Unit u covers (t = u % 4, c2 = u // 4).
    emit_q(0, 0)
    emit_k(0, 0)
    emit_vbb()

    osc_prev = {}
    for u in range(8):
        t, c2 = u % 4, u // 4
        ex = {}
        if u == 0:
            # V emissions trail their xT/wq loads (PV lag 4 covers them)
            ex[2] = lambda: (emit_v(0), emit_v(1))
            ex[3] = lambda: (emit_v(2), emit_v(3), emit_k(0, 1))
            ex[5] = lambda: (emit_v(4), emit_v(5))
            ex[6] = lambda: (emit_v(6), emit_v(7), emit_k(0, 2))
            ex[8] = lambda: (emit_v(8), emit_v(9), emit_k(0, 3))
            ex[9] = lambda: (emit_v(10), emit_v(11))
            ex[10] = lambda: (emit_v(12), emit_v(13))
            ex[11] = lambda: (emit_v(14), emit_v(15))
            ex[13] = lambda: emit_k(1, 0)
            ex[15] = lambda: emit_k(1, 1)
            ex[17] = lambda: emit_q(1, 0)   # seam filler
        elif u < 4:
            ex[2] = lambda t=t: emit_k(t, 2)
            ex[5] = lambda t=t: emit_k(t, 3)
            if t + 1 < 4:
                ex[9] = lambda t=t: emit_k(t + 1, 0)
                ex[12] = lambda t=t: emit_k(t + 1, 1)
                ex[17] = lambda t=t: emit_q(t + 1, 0)   # seam filler
            else:
                ex[17] = lambda: emit_q(0, 1)           # seam filler
        else:
            if t + 1 < 4:
                ex[9] = lambda t=t: emit_q(t + 1, 1)
            if u == 5:
                ex[5] = lambda: emit_proj(0, 0)
                ex[12] = lambda: emit_proj(1, 0)
            elif u == 6:
                ex[5] = lambda: emit_proj(2, 0)
                ex[12] = lambda: emit_proj(3, 0)
                ex[17] = lambda: emit_proj_start(0, 1, 2)
            elif u == 7:
                ex[6] = lambda: emit_proj_cont(0, 1, 2)
        if u - 1 in osc_prev:
            tp, c2p, oscp = osc_prev.pop(u - 1)
            ex[7 if u < 4 else 3] = (
                lambda tp=tp, c2p=c2p, oscp=oscp: emit_o_transpose(tp, c2p, oscp)
            )
        osc = emit_att_unit(
            t, c2, extras=ex, lag=4 if u == 0 else 2, split_drain=(u == 7)
        )
        osc_prev[u] = (t, c2, osc)

    # tail: last unit's transpose + remaining projections (alternating PSUM
    # rings + drain engines + store queues so everything pipelines)
    tp, c2p, oscp = osc_prev.pop(7)
    emit_o_transpose(tp, c2p, oscp, tag="sg")
    emit_proj_finish(0, 1, 3, drain="act")
    emit_proj(1, 1, tag="qk", drain="dve", store="sw")
    emit_proj(2, 1, tag="sg", drain="act")
    emit_proj(3, 1, tag="qk", drain="dve", store="sw")


def build():
    from contextlib import ExitStack

    nc = bacc.Bacc("TRN2", target_bir_lowering=False, debug=False,
                   num_devices=8)
    xT_d = nc.dram_tensor("xT", [E, N], BF16, kind="ExternalInput").ap()
    wq_d = nc.dram_tensor("wqkvT", [E, 3 * E], BF16, kind="ExternalInput").ap()
    qb_d = nc.dram_tensor("qkvb_col", [3 * E, 1], F32, kind="ExternalInput").ap()
    vb_d = nc.dram_tensor("vb_row", [1, E], BF16, kind="ExternalInput").ap()
    pw_d = nc.dram_tensor("pwT", [E, E], BF16, kind="ExternalInput").ap()
    pb_d = nc.dram_tensor("pb_col", [E, 1], F32, kind="ExternalInput").ap()
    ones_d = nc.dram_tensor("ones_const", [1, 128], BF16, kind="ExternalInput").ap()
    zb_d = nc.dram_tensor("zb_const", [128, 1], F32, kind="ExternalInput").ap()
    idn_d = nc.dram_tensor("idn_const", [128, 128], BF16, kind="ExternalInput").ap()
    out_d = nc.dram_tensor("out", [E, NQ], F32, kind="ExternalOutput").ap()
    dram = (xT_d, wq_d, qb_d, vb_d, pw_d, pb_d, ones_d, zb_d, idn_d, out_d)
    with tile.TileContext(nc) as tc, ExitStack() as ctx:
        emit(nc, tc, ctx, dram)
    nc.compile()
    return nc


def _bf16(a):
    return np.asarray(a, dtype=mybir.dt.np(BF16))


def make_in_maps(x, qkv_w, qkv_b, proj_w, proj_b):
    x = np.asarray(x, np.float32)
    qkv_w = np.asarray(qkv_w, np.float32)
    qkv_b = np.asarray(qkv_b, np.float32)
    proj_w = np.asarray(proj_w, np.float32)
    proj_b = np.asarray(proj_b, np.float32)
    xT_all = np.ascontiguousarray(np.transpose(x, (0, 2, 1)))  # [B, E, N]
    wqkvT = _bf16(np.ascontiguousarray(qkv_w.T))
    pwT = _bf16(np.ascontiguousarray(proj_w.T))
    qb_col = np.ascontiguousarray(qkv_b[:, None])
    vb_row = _bf16(np.ascontiguousarray(qkv_b[None, 1024:1536]))
    pb_col = np.ascontiguousarray(proj_b[:, None])
    in_maps = []
    for c in range(8):
        b, h2 = c >> 1, c & 1
        # rotate so this core's queries are always columns 0:NQ (softmax is
        # invariant to key/value order)
        xr = xT_all[b] if h2 == 0 else np.ascontiguousarray(
            np.concatenate([xT_all[b][:, NQ:], xT_all[b][:, :NQ]], axis=1)
        )
        in_maps.append(
            {
                "xT": _bf16(xr),
                "wqkvT": wqkvT,
                "qkvb_col": qb_col,
                "vb_row": vb_row,
                "pwT": pwT,
                "pb_col": pb_col,
                "ones_const": _bf16(np.ones((1, 128), np.float32)),
                "idn_const": _bf16(np.eye(128, dtype=np.float32)),
                "zb_const": np.zeros((128, 1), np.float32),
            }
        )
    return in_maps


_NC_CACHE = None


def _get_nc():
    global _NC_CACHE
    if _NC_CACHE is None:
        _NC_CACHE = build()
    return _NC_CACHE


def assemble(results):
    out = np.empty((4, 2048, 512), np.float32)
    for c in range(8):
        b, h2 = c >> 1, c & 1
        out[b, h2 * NQ : (h2 + 1) * NQ, :] = results[c]["out"].T
    return out


def kernel(x, qkv_w, qkv_b, proj_w, proj_b, _trace=False):
    nc = _get_nc()
    in_maps = make_in_maps(x, qkv_w, qkv_b, proj_w, proj_b)
    res = run_bass_kernel_spmd(
        nc, in_maps, core_ids=list(range(8)), trace=_trace
    )
    out = assemble(res.results)
    if _trace:
        return out, res
    return out
